# revision 9
# baseline (speedup 1.0000x reference)
"""Multi-head causal attention with RoPE on 8 Trainium2 NeuronCores.

Sharding: batch x head-group. Core c owns batch c//4 and heads
[4g, 4g+4) where g = c % 4. QKV projection is column-sliced per core,
attention is fully local per head, and the output projection is
row-parallel: each core writes a full-shape [T, D] partial (bf16) and the
host sums the 4 partials per batch.

On-device layout: q,k live transposed as [head_dim, T] so score tiles are
S^T[k, q], softmax normalization is per-column, and the PV matmul consumes
exp(S^T) directly with v in natural [T, head_dim] layout. All matmuls in
bf16; v carries an extra ones-column so the PV matmul also produces
softmax denominators. Scores accumulate in [128, 1024] PSUM tiles so exp
runs as few, wide ScalarE activations.

Schedule: the two head-pairs are interleaved at q-block granularity
(01j0, 23j0, 01j1, 23j1, ...) so the PE filler load (QKV projection
segments, RoPE, v tiles, output-projection tiles) spreads evenly across
all 80 attention iterations; this keeps the PE dense (HAM clock stays
at 8/8) and moves output tiles + their DMAs off the tail.
"""
import sys

sys.path.insert(0, "/opt/trn_rl_repo")

import numpy as np

B, T, D, H, HD = 2, 2048, 1024, 16, 64
NCORES = 8
GH = 4  # heads per core
DT = 128  # contraction chunk
NDT = D // DT  # 8
KT = 128  # k-tile (score partition dim)
NKT = T // KT  # 16
QC = 1024  # q-chunk width (score free dim / psum tile width)
NQC = T // QC  # 2

# wqkv column layout (prefix-critical first): q01 | k01 | v | q23 | k23
CQ01, CK01, CV, CQ23, CK23 = 0, 128, 256, 512, 640

_CACHE = {}


def _build():
    import concourse.bass as bass  # noqa: F401
    from concourse import bacc
    import concourse.mybir as mybir
    from concourse.tile import TileContext

    F32 = mybir.dt.float32
    BF16 = mybir.dt.bfloat16
    AF = mybir.ActivationFunctionType

    nc = bacc.Bacc("TRN2", target_bir_lowering=False)

    XT = nc.dram_tensor("xt", [D, T], BF16, kind="ExternalInput")
    WQKV = nc.dram_tensor("wqkv", [D, 768], BF16, kind="ExternalInput")
    WOUT = nc.dram_tensor("wout", [256, D], BF16, kind="ExternalInput")
    COS = nc.dram_tensor("cos2", [128, T], BF16, kind="ExternalInput")
    SIN = nc.dram_tensor("sin2", [128, T], BF16, kind="ExternalInput")
    P2T = nc.dram_tensor("p2t", [128, 128], BF16, kind="ExternalInput")
    TRIMASK = nc.dram_tensor("trimask", [128, 128], BF16, kind="ExternalInput")
    ONESBC = nc.dram_tensor("onesbc", [1, 64], BF16, kind="ExternalInput")
    OUTP = nc.dram_tensor("outp", [T, D], BF16, kind="ExternalOutput")

    with TileContext(nc) as tc:
        with (
            tc.tile_pool(name="const", bufs=1) as cst,
            tc.tile_pool(name="xt", bufs=1) as xtp,
            tc.tile_pool(name="qk", bufs=1) as qkp,
            tc.tile_pool(name="rt", bufs=2) as rtp,
            tc.tile_pool(name="v", bufs=1) as vp,
            tc.tile_pool(name="pt", bufs=8) as ptp,
            tc.tile_pool(name="sm", bufs=2) as smp,
            tc.tile_pool(name="ot", bufs=1) as otp,
            tc.tile_pool(name="os", bufs=3) as osp,
            tc.tile_pool(name="psS", bufs=2, space="PSUM") as psS,
            tc.tile_pool(name="psPV", bufs=2, space="PSUM") as psPV,
            tc.tile_pool(name="psA", bufs=2, space="PSUM") as psA,
        ):
            # ---- DMA staging, critical-path first ----
            # sync (HWDGE): p2t (warmup gate), xt quarter-0 evens, trimask,
            #   xt remainder d4-7, wout
            # gpsimd (SWDGE): xt quarter-0 odds, xt remainder d0-3, (outputs)
            # scalar (HWDGE): cos/sin seg0, wqkv wave1 [0:512] (q01,k01,v),
            #   cos/sin rest, wqkv wave2 [512:768]
            p2t = cst.tile([128, 128], BF16, tag="p2t")
            nc.sync.dma_start(p2t[:], P2T[:])

            xt = []
            for d in range(NDT):
                t_ = xtp.tile([DT, T], BF16, tag=f"xt{d}", name=f"xt{d}")
                xt.append(t_)
            # quarter 0 (token cols 0:512) - everything the prefix consumes
            for d in range(NDT):
                eng = nc.sync if d % 2 == 0 else nc.gpsimd
                eng.dma_start(xt[d][:, 0:512], XT[d * DT : (d + 1) * DT, 0:512])

            cos = cst.tile([128, T], BF16, tag="cos")
            sin = cst.tile([128, T], BF16, tag="sin")
            nc.scalar.dma_start(cos[:, 0:512], COS[:, 0:512])
            nc.scalar.dma_start(sin[:, 0:512], SIN[:, 0:512])

            wqkv = []
            for d in range(NDT):
                t_ = cst.tile([DT, 768], BF16, tag=f"wqkv{d}", name=f"wqkv{d}")
                nc.scalar.dma_start(t_[:, 0:512], WQKV[d * DT : (d + 1) * DT, 0:512])
                wqkv.append(t_)

            trimask = cst.tile([128, 128], BF16, tag="trimask")
            onesbc = cst.tile([1, 64], BF16, tag="onesbc")
            nc.sync.dma_start(trimask[:], TRIMASK[:])
            nc.sync.dma_start(onesbc[:], ONESBC[:])

            # xt remainder (token cols 512:2048), one chunk per d
            for d in range(NDT):
                eng = nc.gpsimd if d < 4 else nc.sync
                eng.dma_start(
                    xt[d][:, 512:2048], XT[d * DT : (d + 1) * DT, 512:2048]
                )

            nc.scalar.dma_start(cos[:, 512:2048], COS[:, 512:2048])
            nc.scalar.dma_start(sin[:, 512:2048], SIN[:, 512:2048])

            wout = []
            for g in range(2):
                t_ = cst.tile([128, D], BF16, tag=f"wout{g}", name=f"wout{g}")
                nc.sync.dma_start(t_[:], WOUT[g * 128 : (g + 1) * 128, :])
                wout.append(t_)
            for d in range(NDT):
                nc.scalar.dma_start(
                    wqkv[d][:, 512:768], WQKV[d * DT : (d + 1) * DT, 512:768]
                )

            # ---- HAM warm-up: dummy matmuls keep the PE busy while x DMAs
            # land, so the clock gate is at 8/8 when real work starts ----
            with nc.named_scope("warmup"):
                wps = psS.tile([128, QC], F32, tag="big", name="warm")
                for _ in range(44):
                    nc.tensor.matmul(
                        wps[:, 0:128], p2t[:], p2t[:], start=True, stop=True
                    )

            qk = {}

            # 512-col slice of a q/k projection as one filler chain
            def make_proj_seg(name, col0, s):
                def emit():
                    if qk.get(name) is None:
                        qk[name] = qkp.tile([128, T], BF16, tag=name, name=name)
                    dst = qk[name]
                    ps = psA.tile([128, 512], F32, tag="aux", name=f"{name}s{s}")
                    for d in range(NDT):
                        nc.tensor.matmul(
                            ps[:],
                            wqkv[d][:, col0 : col0 + 128],
                            xt[d][:, s * 512 : (s + 1) * 512],
                            start=(d == 0),
                            stop=(d == NDT - 1),
                        )
                    nc.vector.tensor_copy(dst[:, s * 512 : (s + 1) * 512], ps[:])
                return emit

            def make_rope_seg(name, s):
                def emit():
                    raw = qk[name]
                    sl = slice(s * 512, (s + 1) * 512)
                    psr = psA.tile([128, 512], F32, tag="aux", name=f"r{name}{s}")
                    nc.tensor.matmul(psr[:], p2t[:], raw[:, sl], start=True, stop=True)
                    t1 = rtp.tile([128, 512], BF16, tag="t1s")
                    nc.vector.tensor_mul(t1[:], psr[:], sin[:, sl])
                    t2 = rtp.tile([128, 512], BF16, tag="t2s")
                    nc.vector.tensor_mul(t2[:], raw[:, sl], cos[:, sl])
                    nc.vector.tensor_add(raw[:, sl], t1[:], t2[:])
                return emit

            # ---- v in natural [tok, vdim] layout, plus ones columns ----
            vt = [None] * NKT

            def make_vproj(ti):
                def emit():
                    ps = psA.tile([128, 512], F32, tag="aux", name=f"v{ti}")
                    for d in range(NDT):
                        nc.tensor.matmul(
                            ps[:, 0:256],
                            xt[d][:, ti * KT : (ti + 1) * KT],
                            wqkv[d][:, CV : CV + 256],
                            start=(d == 0),
                            stop=(d == NDT - 1),
                        )
                    v_ = vp.tile([128, 260], BF16, tag=f"v{ti}", name=f"v{ti}")
                    nc.vector.memset(v_[:], 1.0)
                    for h in range(GH):
                        nc.vector.tensor_copy(
                            v_[:, 65 * h : 65 * h + 64], ps[:, 64 * h : 64 * h + 64]
                        )
                    vt[ti] = v_
                return emit

            # ---- prefix: exactly what block 01j0's first iterations need;
            # v2/v3 ride as early fillers inside the block ----
            with nc.named_scope("prefix0"):
                make_proj_seg("q01", CQ01, 0)()
                make_proj_seg("k01", CK01, 0)()
                make_rope_seg("q01", 0)()
                make_rope_seg("k01", 0)()
                make_vproj(0)()
                make_vproj(1)()

            # ---- attention: ot = normalized per-head outputs ----
            ot = [otp.tile([128, T], BF16, tag=f"ot{g}", name=f"ot{g}") for g in range(2)]

            # normalize: row 64 of pso holds sum(exp); fold 1/sum into ot.
            # Deferred so the PE can race ahead into the next block's scores
            # before paying the bcast-matmul dependency on the DVE den copy.
            pending_norm = []

            def flush_norm():
                while pending_norm:
                    h, j, pso = pending_norm.pop(0)
                    pair, hr = h // 2, 64 * (h % 2)
                    den = smp.tile([1, 512], BF16, tag="den")
                    nc.vector.tensor_copy(den[:], pso[64:65, :])
                    psb = psA.tile([128, 512], F32, tag="aux", name=f"bc{h}{j}")
                    nc.tensor.matmul(
                        psb[0:64, :], onesbc[:], den[:], start=True, stop=True
                    )
                    rec = smp.tile([64, 512], F32, tag="rec")
                    nc.vector.reciprocal_approx_fast(rec[:], psb[0:64, :])
                    nc.vector.tensor_mul(
                        ot[pair][hr : hr + 64, j * 512 : (j + 1) * 512],
                        pso[0:64, :],
                        rec[:],
                    )

            def emit_pv(h0, j, pso, n_i, item):
                i, pt, o = item
                for d_, h in ((0, h0), (512, h0 + 1)):
                    nc.tensor.matmul(
                        pso[h][:, o:512],
                        vt[i][:, 65 * h : 65 * h + 65],
                        pt[:, d_ + o : d_ + 512],
                        start=(i == 0),
                        stop=(i == n_i - 1),
                    )

            def attn_block(h0, j, fillers):
                # one q-block (512 wide) of one head pair; both heads share
                # one score tile per k-tile ([o:512] head0, [512+o:1024]
                # head1) so each iteration costs ONE exp.
                pair = h0 // 2
                qT = qk["q01" if pair == 0 else "q23"]
                kT = qk["k01" if pair == 0 else "k23"]
                hr0, hr1 = 0, 64
                n_i = 4 * j + 4
                pso = {
                    h: psPV.tile([65, 512], F32, tag="pv", name=f"pso{h}j{j}")
                    for h in (h0, h0 + 1)
                }
                fifo = []  # software pipeline: PV trails scores by 1 iter
                for i in range(n_i):
                    ob = i * KT - j * 512
                    o = max(0, ob)
                    qsl = slice(j * 512 + o, (j + 1) * 512)
                    ps = psS.tile([128, QC], F32, tag="big", name=f"s{h0}_{j}_{i}")
                    nc.tensor.matmul(
                        ps[:, o:512],
                        kT[hr0 : hr0 + 64, i * KT : (i + 1) * KT],
                        qT[hr0 : hr0 + 64, qsl],
                        start=True,
                        stop=True,
                    )
                    nc.tensor.matmul(
                        ps[:, 512 + o : 1024],
                        kT[hr1 : hr1 + 64, i * KT : (i + 1) * KT],
                        qT[hr1 : hr1 + 64, qsl],
                        start=True,
                        stop=True,
                    )
                    if i == 0:
                        flush_norm()
                    pt = ptp.tile([128, QC], BF16, tag="pt", name=f"pt{h0}_{j}_{i}")
                    # one exp covers both heads; [512:512+o] is stale
                    # psum (bounded, never read downstream)
                    nc.scalar.activation(
                        pt[:, o:QC], ps[:, o:QC], AF.Exp, scale=0.125
                    )
                    if ob >= 0:
                        nc.vector.tensor_mul(
                            pt[:, o : o + 128], pt[:, o : o + 128], trimask[:]
                        )
                        nc.vector.tensor_mul(
                            pt[:, 512 + o : 512 + o + 128],
                            pt[:, 512 + o : 512 + o + 128],
                            trimask[:],
                        )
                    if fillers:
                        fillers.pop(0)()
                        # drain backlogs: pop a second chain when the
                        # remaining iterations can't absorb the list
                        if len(fillers) > n_i - i - 1:
                            fillers.pop(0)()
                    fifo.append((i, pt, o))
                    if len(fifo) > 1:
                        emit_pv(h0, j, pso, n_i, fifo.pop(0))
                while fifo:
                    emit_pv(h0, j, pso, n_i, fifo.pop(0))
                for h in (h0, h0 + 1):
                    pending_norm.append((h, j, pso[h]))

            # output projection for one token tile, split into two 512-col
            # half-chains (each uses one aux PSUM tile). Half 1 DMAs the
            # whole [128, 1024] tile out, alternating sync/gpsimd queues.
            osb_tiles = {}

            def make_oproj_half(ti, half, use_act=False):
                def emit():
                    ps = psA.tile([128, 512], F32, tag="aux", name=f"o{ti}h{half}")
                    for g in range(2):
                        nc.tensor.matmul(
                            ps[:],
                            ot[g][:, ti * KT : (ti + 1) * KT],
                            wout[g][:, half * 512 : (half + 1) * 512],
                            start=(g == 0),
                            stop=(g == 1),
                        )
                    if half == 0:
                        osb_tiles[ti] = osp.tile(
                            [128, D], BF16, tag="ost", name=f"osb{ti}"
                        )
                    osb = osb_tiles[ti]
                    if use_act:
                        nc.scalar.copy(osb[:, half * 512 : (half + 1) * 512], ps[:])
                    else:
                        nc.vector.tensor_copy(
                            osb[:, half * 512 : (half + 1) * 512], ps[:]
                        )
                    if half == 1:
                        eng = nc.sync if ti % 2 == 0 else nc.gpsimd
                        eng.dma_start(OUTP[ti * KT : (ti + 1) * KT, :], osb[:])
                return emit

            # ---- block-interleaved schedule with per-block fillers ----
            # deadlines: q/k seg s of a pair must be roped before that
            # pair's block s starts; v tile 4j..4j+3 before block j's PV;
            # oproj ti needs both pairs' block ti//4 flushed (flush of
            # block X happens at the start of the following block).
            fills = {
                ("01", 0): [
                    make_vproj(2),
                    make_vproj(3),
                    make_proj_seg("q23", CQ23, 0),
                    make_proj_seg("k23", CK23, 0),
                    make_rope_seg("q23", 0),
                    make_rope_seg("k23", 0),
                ],
                ("23", 0): [
                    make_proj_seg("q01", CQ01, 1),
                    make_proj_seg("k01", CK01, 1),
                    make_rope_seg("q01", 1),
                    make_rope_seg("k01", 1),
                ],
                ("01", 1): [
                    make_vproj(4),
                    make_vproj(5),
                    make_vproj(6),
                    make_vproj(7),
                    make_proj_seg("q23", CQ23, 1),
                    make_proj_seg("k23", CK23, 1),
                    make_rope_seg("q23", 1),
                    make_rope_seg("k23", 1),
                ],
                ("23", 1): [
                    make_proj_seg("q01", CQ01, 2),
                    make_proj_seg("k01", CK01, 2),
                    make_rope_seg("q01", 2),
                    make_rope_seg("k01", 2),
                    make_oproj_half(0, 0),
                    make_oproj_half(0, 1),
                    make_oproj_half(1, 0),
                    make_oproj_half(1, 1),
                ],
                ("01", 2): [
                    make_vproj(8),
                    make_vproj(9),
                    make_vproj(10),
                    make_vproj(11),
                    make_proj_seg("q23", CQ23, 2),
                    make_proj_seg("k23", CK23, 2),
                    make_rope_seg("q23", 2),
                    make_rope_seg("k23", 2),
                    make_oproj_half(2, 0),
                    make_oproj_half(2, 1),
                    make_oproj_half(3, 0),
                    make_oproj_half(3, 1),
                ],
                ("23", 2): [
                    make_proj_seg("q01", CQ01, 3),
                    make_proj_seg("k01", CK01, 3),
                    make_rope_seg("q01", 3),
                    make_rope_seg("k01", 3),
                    make_oproj_half(4, 0),
                    make_oproj_half(4, 1),
                    make_oproj_half(5, 0),
                    make_oproj_half(5, 1),
                ],
                ("01", 3): [
                    make_vproj(12),
                    make_vproj(13),
                    make_vproj(14),
                    make_vproj(15),
                    make_proj_seg("q23", CQ23, 3),
                    make_proj_seg("k23", CK23, 3),
                    make_rope_seg("q23", 3),
                    make_rope_seg("k23", 3),
                    make_oproj_half(6, 0),
                    make_oproj_half(6, 1),
                    make_oproj_half(7, 0),
                    make_oproj_half(7, 1),
                ],
                ("23", 3): [
                    make_oproj_half(8, 0),
                    make_oproj_half(8, 1),
                    make_oproj_half(9, 0),
                    make_oproj_half(9, 1),
                    make_oproj_half(10, 0),
                    make_oproj_half(10, 1),
                    make_oproj_half(11, 0),
                    make_oproj_half(11, 1),
                ],
            }

            for j in range(4):
                for pair, h0 in (("01", 0), ("23", 2)):
                    with nc.named_scope(f"attn{pair}j{j}"):
                        fl = fills[(pair, j)]
                        attn_block(h0, j, fl)
                        for f in fl:  # anything the block didn't absorb
                            f()

            # ---- tail: last norms + remaining oproj tiles. A dummy-MM
            # burst bridges the PE-idle gap while the final norm chain
            # (DVE/ACT) runs, so the oproj matmuls stay at full clock ----
            with nc.named_scope("oproj"):
                wps2 = psS.tile([128, QC], F32, tag="big", name="warm2")
                for _ in range(24):
                    nc.tensor.matmul(
                        wps2[:, 0:128], p2t[:], p2t[:], start=True, stop=True
                    )
                flush_norm()
                for ti in range(12, 16):
                    make_oproj_half(ti, 0, use_act=True)()
                    make_oproj_half(ti, 1)()

    nc.compile()
    return nc


def _host_consts(bf16):
    pos = np.arange(T, dtype=np.float64)
    theta = 1.0 / (10000.0 ** (np.arange(0, HD, 2, dtype=np.float64) / HD))
    ang = pos[:, None] * theta[None, :]  # [T, 32]
    cos = np.tile(np.cos(ang), (1, 2)).T  # [64, T]
    sin = np.tile(np.sin(ang), (1, 2)).T
    cos2 = np.vstack([cos, cos]).astype(bf16)  # [128, T] two heads stacked
    sin2 = np.vstack([sin, sin]).astype(bf16)
    # rotate-half as a matmul: rot = P @ q for q in [64, t] column layout
    P = np.zeros((HD, HD), dtype=np.float32)
    for i_ in range(32):
        P[i_, i_ + 32] = -1.0
        P[i_ + 32, i_] = 1.0
    P2 = np.zeros((128, 128), dtype=np.float32)
    P2[0:64, 0:64] = P
    P2[64:128, 64:128] = P
    p2t = np.ascontiguousarray(P2.T).astype(bf16)
    f, p = np.meshgrid(np.arange(128), np.arange(128))
    trimask = (p <= f).astype(bf16)  # [p, f] valid iff p <= f
    onesbc = np.ones((1, 64), dtype=np.float32).astype(bf16)
    return cos2, sin2, p2t, trimask, onesbc


def kernel(x, w_qkv, w_out, b_out):
    import ml_dtypes
    from concourse.bass_utils import run_bass_kernel_spmd

    bf16 = ml_dtypes.bfloat16

    if "nc" not in _CACHE:
        _CACHE["nc"] = _build()
    nc = _CACHE["nc"]

    x = np.asarray(x, dtype=np.float32)
    w_qkv = np.asarray(w_qkv, dtype=np.float32)
    w_out = np.asarray(w_out, dtype=np.float32)
    b_out = np.asarray(b_out, dtype=np.float32)

    cos2, sin2, p2t, trimask, onesbc = _host_consts(bf16)

    wq = w_qkv[:, 0:D]
    wk = w_qkv[:, D : 2 * D]
    wv = w_qkv[:, 2 * D : 3 * D]
    xt_b = [np.ascontiguousarray(x[b].T).astype(bf16) for b in range(B)]

    in_maps = []
    for c in range(NCORES):
        b, g = c // 4, c % 4
        h0 = GH * g  # first head of this core's group
        cs = slice(h0 * HD, h0 * HD + 128)  # heads h0, h0+1
        cs2 = slice(h0 * HD + 128, h0 * HD + 256)  # heads h0+2, h0+3
        vs = slice(h0 * HD, h0 * HD + 256)
        # col layout: q01 | k01 | v | q23 | k23  (prefix-critical first)
        wqkv_c = np.ascontiguousarray(
            np.concatenate([wq[:, cs], wk[:, cs], wv[:, vs], wq[:, cs2], wk[:, cs2]], axis=1)
        ).astype(bf16)  # [D, 768]
        wout_c = np.ascontiguousarray(w_out[vs, :]).astype(bf16)  # [256, D]
        in_maps.append(
            {
                "xt": xt_b[b],
                "wqkv": wqkv_c,
                "wout": wout_c,
                "cos2": cos2,
                "sin2": sin2,
                "p2t": p2t,
                "trimask": trimask,
                "onesbc": onesbc,
            }
        )

    global _last_in_maps
    _last_in_maps = in_maps
    res = run_bass_kernel_spmd(nc, in_maps, list(range(NCORES)))
    out = np.zeros((B, T, D), dtype=np.float64)
    for c in range(NCORES):
        out[c // 4] += np.asarray(res.results[c]["outp"]).astype(np.float64)
    out += b_out.astype(np.float64)
    return out.astype(np.float32)


# revision 15
# speedup vs baseline: 1.0523x; 1.0523x over previous
"""Multi-head causal attention with RoPE on 8 Trainium2 NeuronCores.

Sharding: batch x head-group. Core c owns batch c//4 and heads
[4g, 4g+4) where g = c % 4. QKV projection is column-sliced per core,
attention is fully local per head, and the output projection is
row-parallel: each core writes a full-shape [T, D] partial (bf16) and the
host sums the 4 partials per batch.

On-device layout: q,k live transposed as [head_dim, T] so score tiles are
S^T[k, q], softmax normalization is per-column, and the PV matmul consumes
exp(S^T) directly with v in natural [T, head_dim] layout. All matmuls in
bf16; v carries an extra ones-column so the PV matmul also produces
softmax denominators. Scores accumulate in [128, 1024] PSUM tiles so exp
runs as few, wide ScalarE activations.

Schedule: the two head-pairs are interleaved at q-block granularity
(01j0, 23j0, 01j1, 23j1, ...) so the PE filler load (QKV projection
segments, RoPE, v tiles, output-projection tiles) spreads evenly across
all 80 attention iterations; this keeps the PE dense (HAM clock stays
at 8/8) and moves output tiles + their DMAs off the tail.
"""
import sys

sys.path.insert(0, "/opt/trn_rl_repo")

import numpy as np

B, T, D, H, HD = 2, 2048, 1024, 16, 64
NCORES = 8
GH = 4  # heads per core
DT = 128  # contraction chunk
NDT = D // DT  # 8
KT = 128  # k-tile (score partition dim)
NKT = T // KT  # 16
QC = 1024  # q-chunk width (score free dim / psum tile width)
NQC = T // QC  # 2

# wqkv column layout (prefix-critical first): q01 | k01 | v | q23 | k23
CQ01, CK01, CV, CQ23, CK23 = 0, 128, 256, 512, 640

_CACHE = {}


def _build():
    import concourse.bass as bass  # noqa: F401
    from concourse import bacc
    import concourse.mybir as mybir
    from concourse.tile import TileContext

    F32 = mybir.dt.float32
    BF16 = mybir.dt.bfloat16
    AF = mybir.ActivationFunctionType

    nc = bacc.Bacc("TRN2", target_bir_lowering=False)

    XT = nc.dram_tensor("xt", [D, T], BF16, kind="ExternalInput")
    WQKV = nc.dram_tensor("wqkv", [D, 768], BF16, kind="ExternalInput")
    WOUT = nc.dram_tensor("wout", [256, D], BF16, kind="ExternalInput")
    COS = nc.dram_tensor("cos2", [128, T], BF16, kind="ExternalInput")
    SIN = nc.dram_tensor("sin2", [128, T], BF16, kind="ExternalInput")
    P2T = nc.dram_tensor("p2t", [128, 128], BF16, kind="ExternalInput")
    TRIMASK = nc.dram_tensor("trimask", [128, 128], BF16, kind="ExternalInput")
    ONESBC = nc.dram_tensor("onesbc", [1, 64], BF16, kind="ExternalInput")
    OUTP = nc.dram_tensor("outp", [T, D], BF16, kind="ExternalOutput")

    with TileContext(nc) as tc:
        with (
            tc.tile_pool(name="const", bufs=1) as cst,
            tc.tile_pool(name="xt", bufs=1) as xtp,
            tc.tile_pool(name="qk", bufs=1) as qkp,
            tc.tile_pool(name="rt", bufs=2) as rtp,
            tc.tile_pool(name="v", bufs=1) as vp,
            tc.tile_pool(name="pt", bufs=8) as ptp,
            tc.tile_pool(name="sm", bufs=2) as smp,
            tc.tile_pool(name="ot", bufs=1) as otp,
            tc.tile_pool(name="os", bufs=3) as osp,
            tc.tile_pool(name="psS", bufs=2, space="PSUM") as psS,
            tc.tile_pool(name="psPV", bufs=2, space="PSUM") as psPV,
            tc.tile_pool(name="psA", bufs=2, space="PSUM") as psA,
        ):
            # ---- DMA staging, critical-path first ----
            # sync (HWDGE): p2t (warmup gate), xt quarter-0 evens, trimask,
            #   xt remainder d4-7, wout
            # gpsimd (SWDGE): xt quarter-0 odds, xt remainder d0-3, (outputs)
            # scalar (HWDGE): cos/sin seg0, wqkv wave1 [0:512] (q01,k01,v),
            #   cos/sin rest, wqkv wave2 [512:768]
            p2t = cst.tile([128, 128], BF16, tag="p2t")
            nc.sync.dma_start(p2t[:], P2T[:])

            xt = []
            for d in range(NDT):
                t_ = xtp.tile([DT, T], BF16, tag=f"xt{d}", name=f"xt{d}")
                xt.append(t_)
            # quarter 0 (token cols 0:512) - everything the prefix consumes
            for d in range(NDT):
                eng = nc.sync if d % 2 == 0 else nc.gpsimd
                eng.dma_start(xt[d][:, 0:512], XT[d * DT : (d + 1) * DT, 0:512])

            cos = cst.tile([128, T], BF16, tag="cos")
            sin = cst.tile([128, T], BF16, tag="sin")
            nc.scalar.dma_start(cos[:, 0:512], COS[:, 0:512])
            nc.scalar.dma_start(sin[:, 0:512], SIN[:, 0:512])

            wqkv = []
            for d in range(NDT):
                t_ = cst.tile([DT, 768], BF16, tag=f"wqkv{d}", name=f"wqkv{d}")
                nc.scalar.dma_start(t_[:, 0:512], WQKV[d * DT : (d + 1) * DT, 0:512])
                wqkv.append(t_)

            trimask = cst.tile([128, 128], BF16, tag="trimask")
            nc.sync.dma_start(trimask[:], TRIMASK[:])

            # xt remainder (token cols 512:2048), one chunk per d
            for d in range(NDT):
                eng = nc.gpsimd if d < 4 else nc.sync
                eng.dma_start(
                    xt[d][:, 512:2048], XT[d * DT : (d + 1) * DT, 512:2048]
                )

            nc.scalar.dma_start(cos[:, 512:2048], COS[:, 512:2048])
            nc.scalar.dma_start(sin[:, 512:2048], SIN[:, 512:2048])

            wout = []
            for g in range(2):
                t_ = cst.tile([128, D], BF16, tag=f"wout{g}", name=f"wout{g}")
                nc.sync.dma_start(t_[:], WOUT[g * 128 : (g + 1) * 128, :])
                wout.append(t_)
            for d in range(NDT):
                nc.scalar.dma_start(
                    wqkv[d][:, 512:768], WQKV[d * DT : (d + 1) * DT, 512:768]
                )

            # ---- HAM warm-up: dummy matmuls keep the PE busy while x DMAs
            # land, so the clock gate is at 8/8 when real work starts ----
            with nc.named_scope("warmup"):
                wps = psS.tile([128, QC], F32, tag="big", name="warm")
                for _ in range(44):
                    nc.tensor.matmul(
                        wps[:, 0:128], p2t[:], p2t[:], start=True, stop=True
                    )

            qk = {}

            # 512-col slice of a q/k projection as one filler chain
            def make_proj_seg(name, col0, s):
                def emit():
                    if qk.get(name) is None:
                        qk[name] = qkp.tile([128, T], BF16, tag=name, name=name)
                    dst = qk[name]
                    ps = psA.tile([128, 512], F32, tag="aux", name=f"{name}s{s}")
                    for d in range(NDT):
                        nc.tensor.matmul(
                            ps[:],
                            wqkv[d][:, col0 : col0 + 128],
                            xt[d][:, s * 512 : (s + 1) * 512],
                            start=(d == 0),
                            stop=(d == NDT - 1),
                        )
                    nc.vector.tensor_copy(dst[:, s * 512 : (s + 1) * 512], ps[:])
                return emit

            def make_rope_seg(name, s):
                def emit():
                    raw = qk[name]
                    sl = slice(s * 512, (s + 1) * 512)
                    psr = psA.tile([128, 512], F32, tag="aux", name=f"r{name}{s}")
                    nc.tensor.matmul(psr[:], p2t[:], raw[:, sl], start=True, stop=True)
                    t1 = rtp.tile([128, 512], BF16, tag="t1s")
                    nc.vector.tensor_mul(t1[:], psr[:], sin[:, sl])
                    t2 = rtp.tile([128, 512], BF16, tag="t2s")
                    nc.vector.tensor_mul(t2[:], raw[:, sl], cos[:, sl])
                    nc.vector.tensor_add(raw[:, sl], t1[:], t2[:])
                return emit

            # ---- v in natural [tok, vdim] layout, plus ones columns ----
            vt = [None] * NKT

            def make_vproj(ti):
                def emit():
                    ps = psA.tile([128, 512], F32, tag="aux", name=f"v{ti}")
                    for d in range(NDT):
                        nc.tensor.matmul(
                            ps[:, 0:256],
                            xt[d][:, ti * KT : (ti + 1) * KT],
                            wqkv[d][:, CV : CV + 256],
                            start=(d == 0),
                            stop=(d == NDT - 1),
                        )
                    v_ = vp.tile([128, 260], BF16, tag=f"v{ti}", name=f"v{ti}")
                    nc.vector.memset(v_[:], 1.0)
                    for h in range(GH):
                        nc.vector.tensor_copy(
                            v_[:, 65 * h : 65 * h + 64], ps[:, 64 * h : 64 * h + 64]
                        )
                    vt[ti] = v_
                return emit

            # ---- prefix: exactly what block 01j0's first iterations need;
            # v2/v3 ride as early fillers inside the block ----
            with nc.named_scope("prefix0"):
                make_proj_seg("q01", CQ01, 0)()
                make_proj_seg("k01", CK01, 0)()
                make_rope_seg("q01", 0)()
                make_rope_seg("k01", 0)()
                make_vproj(0)()
                make_vproj(1)()

            # ---- attention: ot = normalized per-head outputs ----
            ot = [otp.tile([128, T], BF16, tag=f"ot{g}", name=f"ot{g}") for g in range(2)]

            # normalize: row 64 of pso holds sum(exp); fold 1/sum into ot.
            # The partition broadcast runs on GpSimd (idle capacity), so the
            # whole chain is off the PE and can flush eagerly at block end.
            def flush_one(h, j, pso):
                pair, hr = h // 2, 64 * (h % 2)
                den = smp.tile([1, 512], F32, tag="den")
                nc.vector.tensor_copy(den[:], pso[64:65, :])
                denb = smp.tile([64, 512], F32, tag="denb")
                nc.gpsimd.partition_broadcast(denb[:], den[:])
                rec = smp.tile([64, 512], F32, tag="rec")
                nc.vector.reciprocal_approx_fast(rec[:], denb[:])
                nc.vector.tensor_mul(
                    ot[pair][hr : hr + 64, j * 512 : (j + 1) * 512],
                    pso[0:64, :],
                    rec[:],
                )

            def emit_pv(h0, j, pso, n_i, item):
                i, pt, o = item
                for d_, h in ((0, h0), (512, h0 + 1)):
                    nc.tensor.matmul(
                        pso[h][:, o:512],
                        vt[i][:, 65 * h : 65 * h + 65],
                        pt[:, d_ + o : d_ + 512],
                        start=(i == 0),
                        stop=(i == n_i - 1),
                    )

            def attn_block(h0, j, fillers):
                # one q-block (512 wide) of one head pair; both heads share
                # one score tile per k-tile ([o:512] head0, [512+o:1024]
                # head1) so each iteration costs ONE exp.
                pair = h0 // 2
                qT = qk["q01" if pair == 0 else "q23"]
                kT = qk["k01" if pair == 0 else "k23"]
                hr0, hr1 = 0, 64
                n_i = 4 * j + 4
                pso = {
                    h: psPV.tile([65, 512], F32, tag="pv", name=f"pso{h}j{j}")
                    for h in (h0, h0 + 1)
                }
                fifo = []  # software pipeline: PV trails scores by 1 iter
                for i in range(n_i):
                    ob = i * KT - j * 512
                    o = max(0, ob)
                    qsl = slice(j * 512 + o, (j + 1) * 512)
                    ps = psS.tile([128, QC], F32, tag="big", name=f"s{h0}_{j}_{i}")
                    nc.tensor.matmul(
                        ps[:, o:512],
                        kT[hr0 : hr0 + 64, i * KT : (i + 1) * KT],
                        qT[hr0 : hr0 + 64, qsl],
                        start=True,
                        stop=True,
                    )
                    nc.tensor.matmul(
                        ps[:, 512 + o : 1024],
                        kT[hr1 : hr1 + 64, i * KT : (i + 1) * KT],
                        qT[hr1 : hr1 + 64, qsl],
                        start=True,
                        stop=True,
                    )
                    pt = ptp.tile([128, QC], BF16, tag="pt", name=f"pt{h0}_{j}_{i}")
                    # one exp covers both heads; [512:512+o] is stale
                    # psum (bounded, never read downstream)
                    nc.scalar.activation(
                        pt[:, o:QC], ps[:, o:QC], AF.Exp, scale=0.125
                    )
                    if ob >= 0:
                        nc.vector.tensor_mul(
                            pt[:, o : o + 128], pt[:, o : o + 128], trimask[:]
                        )
                        nc.vector.tensor_mul(
                            pt[:, 512 + o : 512 + o + 128],
                            pt[:, 512 + o : 512 + o + 128],
                            trimask[:],
                        )
                    if fillers:
                        fillers.pop(0)()
                        # drain backlogs: pop a second chain when the
                        # remaining iterations can't absorb the list
                        if len(fillers) > n_i - i - 1:
                            fillers.pop(0)()
                    fifo.append((i, pt, o))
                    if len(fifo) > 1:
                        emit_pv(h0, j, pso, n_i, fifo.pop(0))
                while fifo:
                    emit_pv(h0, j, pso, n_i, fifo.pop(0))
                for h in (h0, h0 + 1):
                    flush_one(h, j, pso[h])

            # output projection for one token tile, split into two 512-col
            # half-chains (each uses one aux PSUM tile). Half 1 DMAs the
            # whole [128, 1024] tile out, alternating sync/gpsimd queues.
            osb_tiles = {}

            def make_oproj_half(ti, half, use_act=False):
                def emit():
                    ps = psA.tile([128, 512], F32, tag="aux", name=f"o{ti}h{half}")
                    for g in range(2):
                        nc.tensor.matmul(
                            ps[:],
                            ot[g][:, ti * KT : (ti + 1) * KT],
                            wout[g][:, half * 512 : (half + 1) * 512],
                            start=(g == 0),
                            stop=(g == 1),
                        )
                    if half == 0:
                        osb_tiles[ti] = osp.tile(
                            [128, D], BF16, tag="ost", name=f"osb{ti}"
                        )
                    osb = osb_tiles[ti]
                    if use_act:
                        nc.scalar.copy(osb[:, half * 512 : (half + 1) * 512], ps[:])
                    else:
                        nc.vector.tensor_copy(
                            osb[:, half * 512 : (half + 1) * 512], ps[:]
                        )
                    if half == 1:
                        eng = nc.sync if ti % 2 == 0 else nc.gpsimd
                        eng.dma_start(OUTP[ti * KT : (ti + 1) * KT, :], osb[:])
                return emit

            # ---- block-interleaved schedule with per-block fillers ----
            # deadlines: q/k seg s of a pair must be roped before that
            # pair's block s starts; v tile 4j..4j+3 before block j's PV;
            # oproj ti needs both pairs' block ti//4 flushed (flush of
            # block X happens at the start of the following block).
            fills = {
                ("01", 0): [
                    make_vproj(2),
                    make_vproj(3),
                    make_proj_seg("q23", CQ23, 0),
                    make_proj_seg("k23", CK23, 0),
                    make_rope_seg("q23", 0),
                    make_rope_seg("k23", 0),
                ],
                ("23", 0): [
                    make_proj_seg("q01", CQ01, 1),
                    make_proj_seg("k01", CK01, 1),
                    make_rope_seg("q01", 1),
                    make_rope_seg("k01", 1),
                ],
                ("01", 1): [
                    make_vproj(4),
                    make_vproj(5),
                    make_vproj(6),
                    make_vproj(7),
                    make_proj_seg("q23", CQ23, 1),
                    make_proj_seg("k23", CK23, 1),
                    make_rope_seg("q23", 1),
                    make_rope_seg("k23", 1),
                ],
                ("23", 1): [
                    make_proj_seg("q01", CQ01, 2),
                    make_proj_seg("k01", CK01, 2),
                    make_rope_seg("q01", 2),
                    make_rope_seg("k01", 2),
                    make_oproj_half(0, 0),
                    make_oproj_half(0, 1),
                    make_oproj_half(1, 0),
                    make_oproj_half(1, 1),
                ],
                ("01", 2): [
                    make_vproj(8),
                    make_vproj(9),
                    make_vproj(10),
                    make_vproj(11),
                    make_proj_seg("q23", CQ23, 2),
                    make_proj_seg("k23", CK23, 2),
                    make_rope_seg("q23", 2),
                    make_rope_seg("k23", 2),
                    make_oproj_half(2, 0),
                    make_oproj_half(2, 1),
                    make_oproj_half(3, 0),
                    make_oproj_half(3, 1),
                ],
                ("23", 2): [
                    make_proj_seg("q01", CQ01, 3),
                    make_proj_seg("k01", CK01, 3),
                    make_rope_seg("q01", 3),
                    make_rope_seg("k01", 3),
                    make_oproj_half(4, 0),
                    make_oproj_half(4, 1),
                    make_oproj_half(5, 0),
                    make_oproj_half(5, 1),
                ],
                ("01", 3): [
                    make_vproj(12),
                    make_vproj(13),
                    make_vproj(14),
                    make_vproj(15),
                    make_proj_seg("q23", CQ23, 3),
                    make_proj_seg("k23", CK23, 3),
                    make_rope_seg("q23", 3),
                    make_rope_seg("k23", 3),
                    make_oproj_half(6, 0),
                    make_oproj_half(6, 1),
                    make_oproj_half(7, 0),
                    make_oproj_half(7, 1),
                ],
                ("23", 3): [
                    make_oproj_half(8, 0),
                    make_oproj_half(8, 1),
                    make_oproj_half(9, 0),
                    make_oproj_half(9, 1),
                    make_oproj_half(10, 0),
                    make_oproj_half(10, 1),
                    make_oproj_half(11, 0),
                    make_oproj_half(11, 1),
                ],
            }

            for j in range(4):
                for pair, h0 in (("01", 0), ("23", 2)):
                    with nc.named_scope(f"attn{pair}j{j}"):
                        fl = fills[(pair, j)]
                        attn_block(h0, j, fl)
                        for f in fl:  # anything the block didn't absorb
                            f()

            # ---- tail: remaining oproj tiles. A dummy-MM burst bridges the
            # PE-idle gap while the final norm chain (DVE/GpSimd) runs, so
            # the oproj matmuls stay at full clock ----
            with nc.named_scope("oproj"):
                wps2 = psS.tile([128, QC], F32, tag="big", name="warm2")
                for _ in range(20):
                    nc.tensor.matmul(
                        wps2[:, 0:128], p2t[:], p2t[:], start=True, stop=True
                    )
                for ti in range(12, 16):
                    make_oproj_half(ti, 0, use_act=True)()
                    make_oproj_half(ti, 1)()

    nc.compile()
    return nc


def _host_consts(bf16):
    pos = np.arange(T, dtype=np.float64)
    theta = 1.0 / (10000.0 ** (np.arange(0, HD, 2, dtype=np.float64) / HD))
    ang = pos[:, None] * theta[None, :]  # [T, 32]
    cos = np.tile(np.cos(ang), (1, 2)).T  # [64, T]
    sin = np.tile(np.sin(ang), (1, 2)).T
    cos2 = np.vstack([cos, cos]).astype(bf16)  # [128, T] two heads stacked
    sin2 = np.vstack([sin, sin]).astype(bf16)
    # rotate-half as a matmul: rot = P @ q for q in [64, t] column layout
    P = np.zeros((HD, HD), dtype=np.float32)
    for i_ in range(32):
        P[i_, i_ + 32] = -1.0
        P[i_ + 32, i_] = 1.0
    P2 = np.zeros((128, 128), dtype=np.float32)
    P2[0:64, 0:64] = P
    P2[64:128, 64:128] = P
    p2t = np.ascontiguousarray(P2.T).astype(bf16)
    f, p = np.meshgrid(np.arange(128), np.arange(128))
    trimask = (p <= f).astype(bf16)  # [p, f] valid iff p <= f
    onesbc = np.ones((1, 64), dtype=np.float32).astype(bf16)
    return cos2, sin2, p2t, trimask, onesbc


def kernel(x, w_qkv, w_out, b_out):
    import ml_dtypes
    from concourse.bass_utils import run_bass_kernel_spmd

    bf16 = ml_dtypes.bfloat16

    if "nc" not in _CACHE:
        _CACHE["nc"] = _build()
    nc = _CACHE["nc"]

    x = np.asarray(x, dtype=np.float32)
    w_qkv = np.asarray(w_qkv, dtype=np.float32)
    w_out = np.asarray(w_out, dtype=np.float32)
    b_out = np.asarray(b_out, dtype=np.float32)

    cos2, sin2, p2t, trimask, onesbc = _host_consts(bf16)

    wq = w_qkv[:, 0:D]
    wk = w_qkv[:, D : 2 * D]
    wv = w_qkv[:, 2 * D : 3 * D]
    xt_b = [np.ascontiguousarray(x[b].T).astype(bf16) for b in range(B)]

    in_maps = []
    for c in range(NCORES):
        b, g = c // 4, c % 4
        h0 = GH * g  # first head of this core's group
        cs = slice(h0 * HD, h0 * HD + 128)  # heads h0, h0+1
        cs2 = slice(h0 * HD + 128, h0 * HD + 256)  # heads h0+2, h0+3
        vs = slice(h0 * HD, h0 * HD + 256)
        # col layout: q01 | k01 | v | q23 | k23  (prefix-critical first)
        wqkv_c = np.ascontiguousarray(
            np.concatenate([wq[:, cs], wk[:, cs], wv[:, vs], wq[:, cs2], wk[:, cs2]], axis=1)
        ).astype(bf16)  # [D, 768]
        wout_c = np.ascontiguousarray(w_out[vs, :]).astype(bf16)  # [256, D]
        in_maps.append(
            {
                "xt": xt_b[b],
                "wqkv": wqkv_c,
                "wout": wout_c,
                "cos2": cos2,
                "sin2": sin2,
                "p2t": p2t,
                "trimask": trimask,
                "onesbc": onesbc,
            }
        )

    global _last_in_maps
    _last_in_maps = in_maps
    res = run_bass_kernel_spmd(nc, in_maps, list(range(NCORES)))
    out = np.zeros((B, T, D), dtype=np.float64)
    for c in range(NCORES):
        out[c // 4] += np.asarray(res.results[c]["outp"]).astype(np.float64)
    out += b_out.astype(np.float64)
    return out.astype(np.float32)


# revision 16
# speedup vs baseline: 1.0567x; 1.0042x over previous
"""Multi-head causal attention with RoPE on 8 Trainium2 NeuronCores.

Sharding: batch x head-group. Core c owns batch c//4 and heads
[4g, 4g+4) where g = c % 4. QKV projection is column-sliced per core,
attention is fully local per head, and the output projection is
row-parallel: each core writes a full-shape [T, D] partial (bf16) and the
host sums the 4 partials per batch.

On-device layout: q,k live transposed as [head_dim, T] so score tiles are
S^T[k, q], softmax normalization is per-column, and the PV matmul consumes
exp(S^T) directly with v in natural [T, head_dim] layout. All matmuls in
bf16; v carries an extra ones-column so the PV matmul also produces
softmax denominators. Scores accumulate in [128, 1024] PSUM tiles so exp
runs as few, wide ScalarE activations.

Schedule: the two head-pairs are interleaved at q-block granularity
(01j0, 23j0, 01j1, 23j1, ...) so the PE filler load (QKV projection
segments, RoPE, v tiles, output-projection tiles) spreads evenly across
all 80 attention iterations, keeping the PE dense (HAM clock at 8/8).

DMA staging: each queue ring completes only ~one transfer per 2.5us +
bytes/rate (completion receipts serialize), so all inputs are repacked
host-side into [128, N] tensors whose prefix-critical spans are single
contiguous ~1MB DMAs: xt is segment-major (seg0 of all d-chunks first),
wqkv wave-major (q01|k01|v first), cos/sin segment-interleaved.
"""
import sys

sys.path.insert(0, "/opt/trn_rl_repo")

import numpy as np

B, T, D, H, HD = 2, 2048, 1024, 16, 64
NCORES = 8
GH = 4  # heads per core
DT = 128  # contraction chunk
NDT = D // DT  # 8
KT = 128  # k-tile (score partition dim)
NKT = T // KT  # 16
QC = 1024  # q-chunk width (score free dim / psum tile width)
NQC = T // QC  # 2

_CACHE = {}


def _build():
    import concourse.bass as bass  # noqa: F401
    from concourse import bacc
    import concourse.mybir as mybir
    from concourse.tile import TileContext

    F32 = mybir.dt.float32
    BF16 = mybir.dt.bfloat16
    AF = mybir.ActivationFunctionType

    nc = bacc.Bacc("TRN2", target_bir_lowering=False)

    # host-repacked inputs (see kernel() for layouts)
    XTP = nc.dram_tensor("xtp", [128, 4 * NDT * 512], BF16, kind="ExternalInput")
    WQKVP = nc.dram_tensor("wqkvp", [128, 6144], BF16, kind="ExternalInput")
    COSSIN = nc.dram_tensor("cossin", [128, 4096], BF16, kind="ExternalInput")
    CONSTS = nc.dram_tensor("consts", [128, 256], BF16, kind="ExternalInput")
    WOUTP = nc.dram_tensor("woutp", [128, 2048], BF16, kind="ExternalInput")
    OUTP = nc.dram_tensor("outp", [T, D], BF16, kind="ExternalOutput")

    with TileContext(nc) as tc:
        with (
            tc.tile_pool(name="const", bufs=1) as cst,
            tc.tile_pool(name="xt", bufs=1) as xtp,
            tc.tile_pool(name="qk", bufs=1) as qkp,
            tc.tile_pool(name="rt", bufs=2) as rtp,
            tc.tile_pool(name="v", bufs=1) as vp,
            tc.tile_pool(name="pt", bufs=8) as ptp,
            tc.tile_pool(name="sm", bufs=2) as smp,
            tc.tile_pool(name="ot", bufs=1) as otp,
            tc.tile_pool(name="os", bufs=3) as osp,
            tc.tile_pool(name="psS", bufs=2, space="PSUM") as psS,
            tc.tile_pool(name="psPV", bufs=2, space="PSUM") as psPV,
            tc.tile_pool(name="psA", bufs=2, space="PSUM") as psA,
        ):
            # ---- SBUF tiles ----
            consts = cst.tile([128, 256], BF16, tag="consts")
            p2t = consts[:, 0:128]
            trimask = consts[:, 128:256]
            xtall = cst.tile([128, 4 * NDT * 512], BF16, tag="xtall")
            wqkv = cst.tile([128, 6144], BF16, tag="wqkv")
            cossin = cst.tile([128, 4096], BF16, tag="cossin")
            woutall = cst.tile([128, 2048], BF16, tag="woutall")

            # packed-layout accessors
            def xt_seg(d, s):  # [128, 512] token seg s of d-chunk
                c0 = s * (NDT * 512) + d * 512
                return xtall[:, c0 : c0 + 512]

            def xt_cols(d, t0, w):  # w cols starting at token t0 (one seg)
                s, r = divmod(t0, 512)
                c0 = s * (NDT * 512) + d * 512 + r
                return xtall[:, c0 : c0 + w]

            # wqkv wave1: per d [q01(128)|k01(128)|v(256)] at d*512
            # wave2: per d [q23(128)|k23(128)] at 4096 + d*256
            def wq_col(name, d):
                base = {"q01": 0, "k01": 128, "v": 256}.get(name)
                if base is not None:
                    return d * 512 + base
                return 4096 + d * 256 + {"q23": 0, "k23": 128}[name]

            def cos_seg(s):
                return cossin[:, s * 1024 : s * 1024 + 512]

            def sin_seg(s):
                return cossin[:, s * 1024 + 512 : s * 1024 + 1024]

            def wout_cols(g, c0, w):
                return woutall[:, g * 1024 + c0 : g * 1024 + c0 + w]

            # ---- DMA staging: few, large transfers; rings serialize so
            # order = priority. scalar: consts/cos-sin/wave2/wout;
            # sync: wqkv wave1 then xt s2; gpsimd: xt s0, s1, s3 ----
            nc.scalar.dma_start(consts[:], CONSTS[:])
            nc.sync.dma_start(wqkv[:, 0:4096], WQKVP[:, 0:4096])
            nc.gpsimd.dma_start(xtall[:, 0:4096], XTP[:, 0:4096])
            nc.scalar.dma_start(cossin[:, 0:1024], COSSIN[:, 0:1024])
            nc.sync.dma_start(xtall[:, 8192:12288], XTP[:, 8192:12288])
            nc.gpsimd.dma_start(xtall[:, 4096:8192], XTP[:, 4096:8192])
            nc.scalar.dma_start(wqkv[:, 4096:6144], WQKVP[:, 4096:6144])
            nc.gpsimd.dma_start(xtall[:, 12288:16384], XTP[:, 12288:16384])
            nc.scalar.dma_start(cossin[:, 1024:4096], COSSIN[:, 1024:4096])
            nc.scalar.dma_start(woutall[:], WOUTP[:])

            # ---- HAM warm-up: dummy matmuls keep the PE busy while the
            # first DMAs land, so the clock gate is at 8/8 for real work ----
            with nc.named_scope("warmup"):
                wps = psS.tile([128, QC], F32, tag="big", name="warm")
                for _ in range(36):
                    nc.tensor.matmul(
                        wps[:, 0:128], p2t, p2t, start=True, stop=True
                    )

            qk = {}

            # 512-col slice of a q/k projection as one filler chain
            def make_proj_seg(name, s):
                def emit():
                    if qk.get(name) is None:
                        qk[name] = qkp.tile([128, T], BF16, tag=name, name=name)
                    dst = qk[name]
                    ps = psA.tile([128, 512], F32, tag="aux", name=f"{name}s{s}")
                    for d in range(NDT):
                        nc.tensor.matmul(
                            ps[:],
                            wqkv[:, wq_col(name, d) : wq_col(name, d) + 128],
                            xt_seg(d, s),
                            start=(d == 0),
                            stop=(d == NDT - 1),
                        )
                    nc.vector.tensor_copy(dst[:, s * 512 : (s + 1) * 512], ps[:])
                return emit

            def make_rope_seg(name, s):
                def emit():
                    raw = qk[name]
                    sl = slice(s * 512, (s + 1) * 512)
                    psr = psA.tile([128, 512], F32, tag="aux", name=f"r{name}{s}")
                    nc.tensor.matmul(psr[:], p2t, raw[:, sl], start=True, stop=True)
                    t1 = rtp.tile([128, 512], BF16, tag="t1s")
                    nc.vector.tensor_mul(t1[:], psr[:], sin_seg(s))
                    t2 = rtp.tile([128, 512], BF16, tag="t2s")
                    nc.vector.tensor_mul(t2[:], raw[:, sl], cos_seg(s))
                    nc.vector.tensor_add(raw[:, sl], t1[:], t2[:])
                return emit

            # ---- v in natural [tok, vdim] layout, plus ones columns ----
            vt = [None] * NKT

            def make_vproj(ti):
                def emit():
                    ps = psA.tile([128, 512], F32, tag="aux", name=f"v{ti}")
                    for d in range(NDT):
                        nc.tensor.matmul(
                            ps[:, 0:256],
                            xt_cols(d, ti * KT, KT),
                            wqkv[:, wq_col("v", d) : wq_col("v", d) + 256],
                            start=(d == 0),
                            stop=(d == NDT - 1),
                        )
                    v_ = vp.tile([128, 260], BF16, tag=f"v{ti}", name=f"v{ti}")
                    nc.vector.memset(v_[:], 1.0)
                    for h in range(GH):
                        nc.vector.tensor_copy(
                            v_[:, 65 * h : 65 * h + 64], ps[:, 64 * h : 64 * h + 64]
                        )
                    vt[ti] = v_
                return emit

            # ---- prefix: exactly what block 01j0's first iterations need;
            # v2/v3 ride as early fillers inside the block ----
            with nc.named_scope("prefix0"):
                make_proj_seg("q01", 0)()
                make_proj_seg("k01", 0)()
                make_rope_seg("q01", 0)()
                make_rope_seg("k01", 0)()
                make_vproj(0)()
                make_vproj(1)()

            # ---- attention: ot = normalized per-head outputs ----
            ot = [otp.tile([128, T], BF16, tag=f"ot{g}", name=f"ot{g}") for g in range(2)]

            # normalize: row 64 of pso holds sum(exp); fold 1/sum into ot.
            # The partition broadcast runs on GpSimd (idle capacity), so the
            # whole chain is off the PE and can flush eagerly at block end.
            def flush_one(h, j, pso):
                pair, hr = h // 2, 64 * (h % 2)
                den = smp.tile([1, 512], F32, tag="den")
                nc.vector.tensor_copy(den[:], pso[64:65, :])
                denb = smp.tile([64, 512], F32, tag="denb")
                nc.gpsimd.partition_broadcast(denb[:], den[:])
                rec = smp.tile([64, 512], F32, tag="rec")
                nc.vector.reciprocal_approx_fast(rec[:], denb[:])
                nc.vector.tensor_mul(
                    ot[pair][hr : hr + 64, j * 512 : (j + 1) * 512],
                    pso[0:64, :],
                    rec[:],
                )

            def emit_pv(h0, j, pso, n_i, item):
                i, pt, o = item
                for d_, h in ((0, h0), (512, h0 + 1)):
                    nc.tensor.matmul(
                        pso[h][:, o:512],
                        vt[i][:, 65 * h : 65 * h + 65],
                        pt[:, d_ + o : d_ + 512],
                        start=(i == 0),
                        stop=(i == n_i - 1),
                    )

            def attn_block(h0, j, fillers):
                # one q-block (512 wide) of one head pair; both heads share
                # one score tile per k-tile ([o:512] head0, [512+o:1024]
                # head1) so each iteration costs ONE exp.
                pair = h0 // 2
                qT = qk["q01" if pair == 0 else "q23"]
                kT = qk["k01" if pair == 0 else "k23"]
                hr0, hr1 = 0, 64
                n_i = 4 * j + 4
                pso = {
                    h: psPV.tile([65, 512], F32, tag="pv", name=f"pso{h}j{j}")
                    for h in (h0, h0 + 1)
                }
                fifo = []  # software pipeline: PV trails scores by 1 iter
                for i in range(n_i):
                    ob = i * KT - j * 512
                    o = max(0, ob)
                    qsl = slice(j * 512 + o, (j + 1) * 512)
                    ps = psS.tile([128, QC], F32, tag="big", name=f"s{h0}_{j}_{i}")
                    nc.tensor.matmul(
                        ps[:, o:512],
                        kT[hr0 : hr0 + 64, i * KT : (i + 1) * KT],
                        qT[hr0 : hr0 + 64, qsl],
                        start=True,
                        stop=True,
                    )
                    nc.tensor.matmul(
                        ps[:, 512 + o : 1024],
                        kT[hr1 : hr1 + 64, i * KT : (i + 1) * KT],
                        qT[hr1 : hr1 + 64, qsl],
                        start=True,
                        stop=True,
                    )
                    pt = ptp.tile([128, QC], BF16, tag="pt", name=f"pt{h0}_{j}_{i}")
                    # one exp covers both heads; [512:512+o] is stale
                    # psum (bounded, never read downstream)
                    nc.scalar.activation(
                        pt[:, o:QC], ps[:, o:QC], AF.Exp, scale=0.125
                    )
                    if ob >= 0:
                        nc.vector.tensor_mul(
                            pt[:, o : o + 128], pt[:, o : o + 128], trimask
                        )
                        nc.vector.tensor_mul(
                            pt[:, 512 + o : 512 + o + 128],
                            pt[:, 512 + o : 512 + o + 128],
                            trimask,
                        )
                    if fillers:
                        fillers.pop(0)()
                        # drain backlogs: pop a second chain when the
                        # remaining iterations can't absorb the list
                        if len(fillers) > n_i - i - 1:
                            fillers.pop(0)()
                    fifo.append((i, pt, o))
                    if len(fifo) > 1:
                        emit_pv(h0, j, pso, n_i, fifo.pop(0))
                while fifo:
                    emit_pv(h0, j, pso, n_i, fifo.pop(0))
                for h in (h0, h0 + 1):
                    flush_one(h, j, pso[h])

            # output projection for one token tile, split into two 512-col
            # half-chains (each uses one aux PSUM tile). Half 1 DMAs the
            # whole [128, 1024] tile out.
            osb_tiles = {}

            def make_oproj_half(ti, half, use_act=False, dma_eng=None):
                def emit():
                    ps = psA.tile([128, 512], F32, tag="aux", name=f"o{ti}h{half}")
                    for g in range(2):
                        nc.tensor.matmul(
                            ps[:],
                            ot[g][:, ti * KT : (ti + 1) * KT],
                            wout_cols(g, half * 512, 512),
                            start=(g == 0),
                            stop=(g == 1),
                        )
                    if half == 0:
                        osb_tiles[ti] = osp.tile(
                            [128, D], BF16, tag="ost", name=f"osb{ti}"
                        )
                    osb = osb_tiles[ti]
                    if use_act:
                        nc.scalar.copy(osb[:, half * 512 : (half + 1) * 512], ps[:])
                    else:
                        nc.vector.tensor_copy(
                            osb[:, half * 512 : (half + 1) * 512], ps[:]
                        )
                    if half == 1:
                        eng = dma_eng or (nc.sync if ti % 2 == 0 else nc.gpsimd)
                        eng.dma_start(OUTP[ti * KT : (ti + 1) * KT, :], osb[:])
                return emit

            # ---- block-interleaved schedule with per-block fillers ----
            # deadlines: q/k seg s of a pair must be roped before that
            # pair's block s starts; v tile 4j..4j+3 before block j's PV;
            # oproj ti needs both pairs' block ti//4 flushed (flush is
            # eager at block end).
            fills = {
                ("01", 0): [
                    make_vproj(2),
                    make_vproj(3),
                    make_proj_seg("q23", 0),
                    make_proj_seg("k23", 0),
                    make_rope_seg("q23", 0),
                    make_rope_seg("k23", 0),
                ],
                ("23", 0): [
                    make_proj_seg("q01", 1),
                    make_proj_seg("k01", 1),
                    make_rope_seg("q01", 1),
                    make_rope_seg("k01", 1),
                ],
                ("01", 1): [
                    make_vproj(4),
                    make_vproj(5),
                    make_vproj(6),
                    make_vproj(7),
                    make_proj_seg("q23", 1),
                    make_proj_seg("k23", 1),
                    make_rope_seg("q23", 1),
                    make_rope_seg("k23", 1),
                ],
                ("23", 1): [
                    make_proj_seg("q01", 2),
                    make_proj_seg("k01", 2),
                    make_rope_seg("q01", 2),
                    make_rope_seg("k01", 2),
                    make_oproj_half(0, 0),
                    make_oproj_half(0, 1),
                    make_oproj_half(1, 0),
                    make_oproj_half(1, 1),
                ],
                ("01", 2): [
                    make_vproj(8),
                    make_vproj(9),
                    make_vproj(10),
                    make_vproj(11),
                    make_proj_seg("q23", 2),
                    make_proj_seg("k23", 2),
                    make_rope_seg("q23", 2),
                    make_rope_seg("k23", 2),
                    make_oproj_half(2, 0),
                    make_oproj_half(2, 1),
                    make_oproj_half(3, 0),
                    make_oproj_half(3, 1),
                ],
                ("23", 2): [
                    make_proj_seg("q01", 3),
                    make_proj_seg("k01", 3),
                    make_rope_seg("q01", 3),
                    make_rope_seg("k01", 3),
                    make_oproj_half(4, 0),
                    make_oproj_half(4, 1),
                    make_oproj_half(5, 0),
                    make_oproj_half(5, 1),
                ],
                ("01", 3): [
                    make_vproj(12),
                    make_vproj(13),
                    make_vproj(14),
                    make_vproj(15),
                    make_proj_seg("q23", 3),
                    make_proj_seg("k23", 3),
                    make_rope_seg("q23", 3),
                    make_rope_seg("k23", 3),
                    make_oproj_half(6, 0),
                    make_oproj_half(6, 1),
                    make_oproj_half(7, 0),
                    make_oproj_half(7, 1),
                ],
                ("23", 3): [
                    make_oproj_half(8, 0),
                    make_oproj_half(8, 1),
                    make_oproj_half(9, 0),
                    make_oproj_half(9, 1),
                    make_oproj_half(10, 0),
                    make_oproj_half(10, 1),
                    make_oproj_half(11, 0),
                    make_oproj_half(11, 1),
                ],
            }

            for j in range(4):
                for pair, h0 in (("01", 0), ("23", 2)):
                    with nc.named_scope(f"attn{pair}j{j}"):
                        fl = fills[(pair, j)]
                        attn_block(h0, j, fl)
                        for f in fl:  # anything the block didn't absorb
                            f()

            # ---- tail: remaining oproj tiles. A dummy-MM burst bridges the
            # PE-idle gap while the final norm chain (DVE/GpSimd) runs, so
            # the oproj matmuls stay at full clock. The four output DMAs
            # spread across all three queues ----
            with nc.named_scope("oproj"):
                wps2 = psS.tile([128, QC], F32, tag="big", name="warm2")
                for _ in range(20):
                    nc.tensor.matmul(
                        wps2[:, 0:128], p2t, p2t, start=True, stop=True
                    )
                tail_eng = [nc.sync, nc.gpsimd, nc.scalar, nc.sync]
                for ti in range(12, 16):
                    make_oproj_half(ti, 0, use_act=True)()
                    make_oproj_half(ti, 1, dma_eng=tail_eng[ti - 12])()

    nc.compile()
    return nc


def _host_consts(bf16):
    pos = np.arange(T, dtype=np.float64)
    theta = 1.0 / (10000.0 ** (np.arange(0, HD, 2, dtype=np.float64) / HD))
    ang = pos[:, None] * theta[None, :]  # [T, 32]
    cos = np.tile(np.cos(ang), (1, 2)).T  # [64, T]
    sin = np.tile(np.sin(ang), (1, 2)).T
    cos2 = np.vstack([cos, cos]).astype(bf16)  # [128, T] two heads stacked
    sin2 = np.vstack([sin, sin]).astype(bf16)
    # rotate-half as a matmul: rot = P @ q for q in [64, t] column layout
    P = np.zeros((HD, HD), dtype=np.float32)
    for i_ in range(32):
        P[i_, i_ + 32] = -1.0
        P[i_ + 32, i_] = 1.0
    P2 = np.zeros((128, 128), dtype=np.float32)
    P2[0:64, 0:64] = P
    P2[64:128, 64:128] = P
    p2t = np.ascontiguousarray(P2.T).astype(bf16)
    f, p = np.meshgrid(np.arange(128), np.arange(128))
    trimask = (p <= f).astype(bf16)  # [p, f] valid iff p <= f
    # segment-interleaved cos/sin: [cos_s0|sin_s0|cos_s1|sin_s1|...]
    cosr = cos2.reshape(128, 4, 512)
    sinr = sin2.reshape(128, 4, 512)
    cossin = np.ascontiguousarray(
        np.concatenate([cosr, sinr], axis=2).reshape(128, 4096)
    )
    consts = np.ascontiguousarray(np.concatenate([p2t, trimask], axis=1))
    return cossin, consts


def kernel(x, w_qkv, w_out, b_out):
    import ml_dtypes
    from concourse.bass_utils import run_bass_kernel_spmd

    bf16 = ml_dtypes.bfloat16

    if "nc" not in _CACHE:
        _CACHE["nc"] = _build()
    nc = _CACHE["nc"]

    x = np.asarray(x, dtype=np.float32)
    w_qkv = np.asarray(w_qkv, dtype=np.float32)
    w_out = np.asarray(w_out, dtype=np.float32)
    b_out = np.asarray(b_out, dtype=np.float32)

    cossin, consts = _host_consts(bf16)

    wq = w_qkv[:, 0:D]
    wk = w_qkv[:, D : 2 * D]
    wv = w_qkv[:, 2 * D : 3 * D]
    # xt packed segment-major: xtp[p, s*4096 + d*512 + c] = x.T[d*128+p, s*512+c]
    xt_b = [
        np.ascontiguousarray(
            x[b].T.reshape(NDT, 128, 4, 512)
            .transpose(1, 2, 0, 3)
            .reshape(128, 4 * NDT * 512)
        ).astype(bf16)
        for b in range(B)
    ]

    in_maps = []
    for c in range(NCORES):
        b, g = c // 4, c % 4
        h0 = GH * g  # first head of this core's group
        cs = slice(h0 * HD, h0 * HD + 128)  # heads h0, h0+1
        cs2 = slice(h0 * HD + 128, h0 * HD + 256)  # heads h0+2, h0+3
        vs = slice(h0 * HD, h0 * HD + 256)
        # wave1 per d: [q01|k01|v] (512 cols), wave2 per d: [q23|k23] (256)
        w1 = np.concatenate([wq[:, cs], wk[:, cs], wv[:, vs]], axis=1)  # [D,512]
        w2 = np.concatenate([wq[:, cs2], wk[:, cs2]], axis=1)  # [D,256]
        w1p = w1.reshape(NDT, 128, 512).transpose(1, 0, 2).reshape(128, 4096)
        w2p = w2.reshape(NDT, 128, 256).transpose(1, 0, 2).reshape(128, 2048)
        wqkv_p = np.ascontiguousarray(np.concatenate([w1p, w2p], axis=1)).astype(bf16)
        wout_p = np.ascontiguousarray(
            w_out[vs, :].reshape(2, 128, D).transpose(1, 0, 2).reshape(128, 2 * D)
        ).astype(bf16)
        in_maps.append(
            {
                "xtp": xt_b[b],
                "wqkvp": wqkv_p,
                "cossin": cossin,
                "consts": consts,
                "woutp": wout_p,
            }
        )

    global _last_in_maps
    _last_in_maps = in_maps
    res = run_bass_kernel_spmd(nc, in_maps, list(range(NCORES)))
    out = np.zeros((B, T, D), dtype=np.float64)
    for c in range(NCORES):
        out[c // 4] += np.asarray(res.results[c]["outp"]).astype(np.float64)
    out += b_out.astype(np.float64)
    return out.astype(np.float32)
